# revision 2
# baseline (speedup 1.0000x reference)
"""Trainium2 Bass kernel for nn_EncodingLayer (spiking encoder).

Computes, for x:[B,S,I,H] and encoding:[I,H]:
    cur = einsum("bsih,ih->bsh", x, encoding)            # [B,S,H]
    then a 320-step LIF scan (5 substeps per s, alpha=0.9, soft reset,
    Heaviside spikes) producing z:[B, S*5, H].

Strategy: data-parallel over B across 8 NeuronCores (2 batches/core).
Per core:
  - x tiles [I=128, H=512] DMA'd in 1 MB chunks (4 sequence positions).
  - einsum reduction over I via two engine paths (load-balanced):
      * PE path: fused fp32 matmul  x_chunk[128,128]^T @ ones[128,1]
        -> psum column = cur^T column.
      * PE-transpose + DVE path: exact fp32 PE transposes then a single
        strided DVE tensor_reduce along the free axis.
    Both land cur in "scan layout": partition p = h%128,
    column = s*8 + b*4 + h//128.
  - scan: 2 fused DVE scalar_tensor_tensor ops per step on [128, 8] state
    (state kept negated so (in0 op0 s) op1 in1 covers the whole update):
        u_t = (w * -alpha) + cur_s        # u = true membrane potential
        w   = (u_t is_gt 1) - u_t         # w = z - u = -v_next
    u_t is stored; spikes are extracted en masse: z = (u > 1).
  - output: PE transposes of z back to [timestep, H] rows, DMA out.
"""

import sys
import numpy as np

for _p in ("/opt/trn_rl_repo", "/root/.axon_site/_ro/trn_rl_repo"):
    if _p not in sys.path:
        sys.path.append(_p)

import concourse.bass as bass
import concourse.mybir as mybir
import concourse.tile as tile_mod
from concourse.tile import TileContext
from concourse.masks import make_identity
from concourse.vector_clock import ScopedClock
from concourse.bass_utils import run_bass_kernel_spmd

F32 = mybir.dt.float32
OP = mybir.AluOpType
AX = mybir.AxisListType

NUM_TIMESTEPS = 5
ALPHA = 0.9
THRESHOLD = 1.0

B, S, I, H = 16, 64, 128, 512
NCORES = 8
BPC = B // NCORES          # batches per core = 2
ST = S * NUM_TIMESTEPS     # 320
SBLK = 4                   # sequence positions per DMA chunk (1 MB)
NBLK = S // SBLK           # 16
PE_BLOCKS = 9              # einsum blocks on the PE-matmul path (tunable 0..16)


# ---------------------------------------------------------------------------
# Workaround: this walrus build accepts at most ONE sync-wait command per
# instruction.  Split multi-sem waits into single-wait nops.
# ---------------------------------------------------------------------------
_orig_commit = tile_mod.TileContext._commit_instruction


def _patched_commit(self, inst, lazy_reg_writes: bool = True):
    si = getattr(inst, "sync_info", None)
    if (
        si is not None
        and si.on_wait
        and len(si.on_wait) > 1
        and inst.engine != mybir.EngineType.Unassigned
    ):
        waits = list(si.on_wait)
        inst.sync_info = mybir.SyncInfo(on_wait=waits[:1], on_update=list(si.on_update))
        for w in waits[1:]:
            nop = mybir.InstNoOp(
                name=self.nc.get_next_instruction_name(),
                sync_info=mybir.SyncInfo(on_wait=[w], on_update=[]),
                bass_nofuse=True,
                engine=inst.engine,
                text_hint="split_wait",
            )
            _orig_commit(self, nop, lazy_reg_writes=False)
    return _orig_commit(self, inst, lazy_reg_writes)


def _patched_drain_and_barrier(self, tick_clock, wait_clock):
    drain_inst = self.nc.sync.drain()
    wait_clock.add_sem_waits(
        drain_inst.ins, ScopedClock({None: tick_clock.global_clock})
    )
    si = drain_inst.ins.sync_info
    waits = list(si.on_wait) if si is not None else []
    if len(waits) > 1:
        drain_inst.ins.sync_info = mybir.SyncInfo(
            on_wait=waits[:1], on_update=list(si.on_update)
        )
        for w in waits[1:]:
            nop_inst = self.nc.sync.nop(nofuse=True, hint="split_drain_wait")
            nop_inst.ins.sync_info = mybir.SyncInfo(on_wait=[w], on_update=[])
    self.nc.all_engine_barrier()
    popped = self.nc._tile_sem_poison_stack.pop()
    assert popped is self._sem_poison
    self.nc.clear_and_free_semaphores(list(self.sems.allocated().values()))
    self.nc.all_engine_barrier()


if getattr(tile_mod.TileContext, "_ant_wait_split_patch", False) is False:
    tile_mod.TileContext._commit_instruction = _patched_commit
    tile_mod.TileContext._drain_and_barrier = _patched_drain_and_barrier
    tile_mod.TileContext._ant_wait_split_patch = True


# ---------------------------------------------------------------------------
# Kernel builder (per-core program; pure SPMD data parallel, no collectives)
# ---------------------------------------------------------------------------
def build_kernel(ones_encoding: bool):
    nc = bass.Bass(target_bir_lowering=False)
    x_in = nc.declare_dram_parameter("x", [BPC, S, I, H], F32, isOutput=False)
    if not ones_encoding:
        enc_in = nc.declare_dram_parameter("encoding", [I, H], F32, isOutput=False)
    y_out = nc.declare_dram_parameter("y", [BPC, ST, H], F32, isOutput=True)

    with TileContext(nc) as tc:
        with tc.tile_pool(name="const", bufs=1) as constp, \
             tc.tile_pool(name="xp", bufs=6) as xp, \
             tc.tile_pool(name="curp", bufs=NBLK) as curp, \
             tc.tile_pool(name="scanp", bufs=1) as scanp, \
             tc.tile_pool(name="outp", bufs=2) as outp, \
             tc.tile_pool(name="pe_cur_p", bufs=1, space="PSUM") as pe_cur_p, \
             tc.tile_pool(name="tpp", bufs=4, space="PSUM") as tpp, \
             tc.tile_pool(name="opp", bufs=2, space="PSUM") as opp:

            ident = constp.tile([128, 128], F32, name="ident")
            make_identity(nc, ident)
            ones = constp.tile([128, 1], F32, name="ones")
            nc.vector.memset(ones, 1.0)
            if not ones_encoding:
                enc = constp.tile([I, H], F32, name="enc")
                nc.sync.dma_start(out=enc, in_=enc_in[:])

            # cur in scan layout, one tile per 4-s block: [p=h%128, 32 cols]
            cur_blks = [
                curp.tile([128, SBLK * 2 * 4], F32, name=f"cur{blk}")
                for blk in range(NBLK)
            ]
            # psum bank collecting PE-path cur columns (512 = 64s * 2b * 4hc)
            pe_cur = pe_cur_p.tile([128, 512], F32, name="pe_cur")

            # scan state and u storage (5 tiles of 64 timesteps each)
            w = scanp.tile([128, 8], F32, name="w")
            nc.vector.memset(w, 0.0)
            u_tiles = [
                scanp.tile([128, 512], F32, name=f"u{q}") for q in range(5)
            ]
            # z storage grouped by output chunk (128/128/64 timesteps)
            z_tiles = [
                scanp.tile([128, 1024], F32, name="z0"),
                scanp.tile([128, 1024], F32, name="z1"),
                scanp.tile([128, 512], F32, name="z2"),
            ]

            def emit_einsum_block(blk):
                s0 = blk * SBLK
                xts = {}
                for b in range(BPC):
                    xt = xp.tile([128, SBLK * H], F32, name="xt", tag=f"x{b}")
                    nc.sync.dma_start(
                        out=xt.rearrange("p (si h) -> p si h", h=H),
                        in_=x_in[b, s0 : s0 + SBLK].rearrange("si i h -> i si h"),
                    )
                    if not ones_encoding:
                        xe = xp.tile([128, SBLK * H], F32, name="xe", tag=f"xe{b}")
                        for si in range(SBLK):
                            nc.vector.tensor_tensor(
                                out=xe[:, si * H : (si + 1) * H],
                                in0=xt[:, si * H : (si + 1) * H],
                                in1=enc,
                                op=OP.mult,
                            )
                        xt = xe
                    xts[b] = xt

                if blk < PE_BLOCKS:
                    # PE fused-matmul path: one psum column per (s, b, hc)
                    for si in range(SBLK):
                        s = s0 + si
                        for b in range(BPC):
                            for hc in range(4):
                                col = s * 8 + b * 4 + hc
                                nc.tensor.matmul(
                                    pe_cur[:, col : col + 1],
                                    lhsT=xts[b][:, si * H + hc * 128 : si * H + (hc + 1) * 128],
                                    rhs=ones,
                                    start=True,
                                    stop=True,
                                )
                    nc.vector.tensor_copy(
                        cur_blks[blk], pe_cur[:, s0 * 8 : (s0 + SBLK) * 8]
                    )
                else:
                    # PE transpose + DVE strided reduce path
                    for si in range(SBLK):
                        s = s0 + si
                        for b in range(BPC):
                            ps = tpp.tile([128, 512], F32, name="tp", tag="tp")
                            for hc in range(4):
                                nc.tensor.transpose(
                                    ps[:, hc * 128 : (hc + 1) * 128],
                                    xts[b][:, si * H + hc * 128 : si * H + (hc + 1) * 128],
                                    ident,
                                )
                            nc.vector.tensor_reduce(
                                out=cur_blks[blk][:, si * 8 + b * 4 : si * 8 + b * 4 + 4],
                                in_=ps.rearrange("p (c i) -> p c i", c=4),
                                axis=AX.X,
                                op=OP.add,
                            )

            def emit_scan_block(blk):
                for si in range(SBLK):
                    s = blk * SBLK + si
                    i_t = cur_blks[blk][:, si * 8 : (si + 1) * 8]
                    for t in range(NUM_TIMESTEPS):
                        st = s * NUM_TIMESTEPS + t
                        u_slice = u_tiles[st // 64][:, (st % 64) * 8 : (st % 64) * 8 + 8]
                        nc.vector.scalar_tensor_tensor(
                            out=u_slice, in0=w, scalar=-ALPHA, in1=i_t,
                            op0=OP.mult, op1=OP.add,
                        )
                        nc.vector.scalar_tensor_tensor(
                            out=w, in0=u_slice, scalar=THRESHOLD, in1=u_slice,
                            op0=OP.is_gt, op1=OP.subtract,
                        )

            # software pipeline: einsum(blk) runs ahead, scan lags 2 blocks
            LAG = 2
            for blk in range(NBLK):
                emit_einsum_block(blk)
                if blk >= LAG:
                    emit_scan_block(blk - LAG)
            for blk in range(NBLK - LAG, NBLK):
                emit_scan_block(blk)

            # spikes: z = (u > 1), en masse; u tile q covers st in [64q, 64q+64)
            for q in range(5):
                zt = z_tiles[q // 2]
                off = (q % 2) * 512
                nc.vector.tensor_scalar(
                    out=zt[:, off : off + 512], in0=u_tiles[q],
                    scalar1=THRESHOLD, scalar2=None, op0=OP.is_gt,
                )

            # output: transpose z chunks back to [timestep, H] rows and store
            for b in range(BPC):
                for ci, (st0, chunk) in enumerate(((0, 128), (128, 128), (256, 64))):
                    po = opp.tile([128, 512], F32, name="po", tag="po")
                    zt = z_tiles[ci]
                    z3 = zt.rearrange("p (st f) -> p st f", f=8)
                    for hc in range(4):
                        nc.tensor.transpose(
                            po[0:chunk, hc * 128 : (hc + 1) * 128],
                            z3[:, :, b * 4 + hc][:, 0:chunk],
                            ident,
                        )
                    osb = outp.tile([128, 512], F32, name="osb", tag="osb")
                    nc.vector.tensor_copy(osb[0:chunk, :], po[0:chunk, :])
                    nc.sync.dma_start(
                        out=y_out[b, st0 : st0 + chunk, :], in_=osb[0:chunk, :]
                    )

    return nc


_KERNEL_CACHE = {}


def _get_kernel(ones_encoding: bool):
    if ones_encoding not in _KERNEL_CACHE:
        _KERNEL_CACHE[ones_encoding] = build_kernel(ones_encoding)
    return _KERNEL_CACHE[ones_encoding]


def kernel(x: np.ndarray, encoding: np.ndarray) -> np.ndarray:
    x = np.ascontiguousarray(x, dtype=np.float32)
    encoding = np.ascontiguousarray(encoding, dtype=np.float32)
    assert x.shape == (B, S, I, H), x.shape
    assert encoding.shape == (I, H), encoding.shape

    ones_encoding = bool(np.all(encoding == 1.0))
    nc = _get_kernel(ones_encoding)

    xs = x.reshape(NCORES, BPC, S, I, H)
    in_maps = []
    for c in range(NCORES):
        m = {"x": xs[c]}
        if not ones_encoding:
            m["encoding"] = encoding
        in_maps.append(m)

    res = run_bass_kernel_spmd(nc, in_maps, list(range(NCORES)))
    y = np.concatenate([res.results[c]["y"] for c in range(NCORES)], axis=0)
    return y.astype(np.float32)


if __name__ == "__main__":
    rng = np.random.default_rng(0)
    x = rng.standard_normal((B, S, I, H), dtype=np.float32)
    enc = np.ones((I, H), np.float32)
    y = kernel(x, enc)
    print("y", y.shape, y.dtype, y.mean())
